# revision 25
# baseline (speedup 1.0000x reference)
"""GQA attention layer (B=2,S=2048,D=2048,H=16,KV=4,HD=128) on 8 trn2 cores.

Sharding: core = (b, g) for b in {0,1} (batch), g in {0..3} (kv group).
Each core computes q-heads 4g..4g+3 + kv head g for batch b, producing a
partial o-projection [S, D]; the host sums the 4 partials per batch.

Per-core kernel: everything in transposed layout (head_dim on partitions),
bf16 matmuls with fp32 accumulation, softmax without max-subtraction
(logits bounded after RMSNorm), causal block skipping. Partition-dim
reductions (rms-norm sum-of-squares, softmax denominator) via ones-matmul
with M=128 so the result is already broadcast across partitions;
reciprocals/rsqrt computed on ScalarE as exp(-a*ln(x)).
"""
import numpy as np
import ml_dtypes

B, S, DM = 2, 2048, 2048
H, KV, HD = 16, 4, 128
G = H // KV
THETA = 10000.0
EPS = 1e-6

P = 128         # partitions
CH = 512        # s-chunk (matmul N)
NCH = S // CH   # 4
KT = DM // P    # 16 contraction tiles
NST = S // P    # 16 s-tiles

_CACHE = {}
# extra kwargs for run_bass_kernel_spmd (test harness sets trace/tmpdir here)
_RUN_KWARGS = {}


def _build_nc():
    from concourse import bacc, mybir
    import concourse.tile as tile
    from contextlib import ExitStack

    f32 = mybir.dt.float32
    bf16 = mybir.dt.bfloat16
    Act = mybir.ActivationFunctionType

    nc = bacc.Bacc()
    d_xt = nc.declare_dram_parameter("xt", [NCH, P, KT, CH], bf16, isOutput=False)
    d_wq = nc.declare_dram_parameter("wq4", [P, KT, G, HD], bf16, isOutput=False)
    d_wk = nc.declare_dram_parameter("wk1", [P, KT, HD], bf16, isOutput=False)
    d_wv = nc.declare_dram_parameter("wv1", [P, KT, HD], bf16, isOutput=False)
    d_wo = nc.declare_dram_parameter("wo4", [HD, G, DM], bf16, isOutput=False)
    d_qs = nc.declare_dram_parameter("qsc", [HD, 1], f32, isOutput=False)
    d_ks = nc.declare_dram_parameter("ksc", [HD, 1], f32, isOutput=False)
    d_cos = nc.declare_dram_parameter("cos_t", [P, S], f32, isOutput=False)
    d_sin = nc.declare_dram_parameter("sin_t", [P, S], f32, isOutput=False)
    d_psw = nc.declare_dram_parameter("psw", [P, P], bf16, isOutput=False)
    d_tri = nc.declare_dram_parameter("tri", [P, P], bf16, isOutput=False)
    d_ident = nc.declare_dram_parameter("ident", [P, P], bf16, isOutput=False)
    d_out = nc.declare_dram_parameter("o_part", [S, DM], f32, isOutput=True)

    with tile.TileContext(nc) as tc, ExitStack() as ctx:
        const = ctx.enter_context(tc.tile_pool(name="const", bufs=1))
        xin = ctx.enter_context(tc.tile_pool(name="xin", bufs=2))
        work = ctx.enter_context(tc.tile_pool(name="work", bufs=4))
        wnorm = ctx.enter_context(tc.tile_pool(name="wnorm", bufs=2))
        # PSUM: 8 banks total, all pools alive all kernel so phases interleave
        p_pa = ctx.enter_context(tc.tile_pool(name="p_pa", bufs=3, space="PSUM"))
        p_sc = ctx.enter_context(tc.tile_pool(name="p_sc", bufs=2, space="PSUM"))
        p_red = ctx.enter_context(tc.tile_pool(name="p_red", bufs=1, space="PSUM"))
        p_acc = ctx.enter_context(tc.tile_pool(name="p_acc", bufs=2, space="PSUM"))

        # ---- persistent SBUF ----
        # first-needed data (wq k-tiles + first x chunk) interleaved on the
        # sync HWDGE ring; bulky later-needed tables go on the ACT ring.
        wq_sb = const.tile([P, KT, G, HD], bf16, tag="wq_sb")
        xt0 = xin.tile([P, KT, CH], bf16, tag="xt_c")
        for i in range(4):
            nc.sync.dma_start(out=wq_sb[:, 4 * i:4 * i + 4], in_=d_wq[:, 4 * i:4 * i + 4])
            nc.scalar.dma_start(out=xt0[:, 4 * i:4 * i + 4], in_=d_xt[0, :, 4 * i:4 * i + 4])
        wk_sb = const.tile([P, KT, HD], bf16, tag="wk_sb")
        nc.sync.dma_start(out=wk_sb, in_=d_wk[:])
        wv_sb = const.tile([P, KT, HD], bf16, tag="wv_sb")
        nc.sync.dma_start(out=wv_sb, in_=d_wv[:])
        qsc_sb = const.tile([HD, 1], f32, tag="qsc_sb")
        nc.sync.dma_start(out=qsc_sb, in_=d_qs[:])
        ksc_sb = const.tile([HD, 1], f32, tag="ksc_sb")
        nc.sync.dma_start(out=ksc_sb, in_=d_ks[:])
        psw_sb = const.tile([P, P], bf16, tag="psw_sb")
        nc.sync.dma_start(out=psw_sb, in_=d_psw[:])
        cos_sb = const.tile([P, S], f32, tag="cos_sb")
        nc.scalar.dma_start(out=cos_sb, in_=d_cos[:])
        sin_sb = const.tile([P, S], f32, tag="sin_sb")
        nc.scalar.dma_start(out=sin_sb, in_=d_sin[:])
        tri_sb = const.tile([P, P], bf16, tag="tri_sb")
        nc.scalar.dma_start(out=tri_sb, in_=d_tri[:])
        wo_sb = const.tile([P, G, DM], bf16, tag="wo_sb")
        nc.scalar.dma_start(out=wo_sb, in_=d_wo[:])
        ident_sb = const.tile([P, P], bf16, tag="ident_sb")
        nc.scalar.dma_start(out=ident_sb, in_=d_ident[:])
        ones_bb = const.tile([P, P], bf16, tag="ones_bb")
        nc.vector.memset(ones_bb, 1.0)
        eps_q = const.tile([P, 1], f32, tag="eps_q")
        nc.vector.memset(eps_q, float(HD * EPS))
        eps_k = const.tile([P, 1], f32, tag="eps_k")
        nc.vector.memset(eps_k, float(EPS))

        # roped q heads / k / v / normalized att, persistent
        qro = [const.tile([P, S], bf16, tag=f"qro{h}", name=f"qro{h}") for h in range(G)]
        kro = const.tile([P, S], bf16, tag="kro")
        v_sb = const.tile([P, NST, HD], bf16, tag="v_sb")
        att_sb = [const.tile([P, S], bf16, tag=f"att{h}", name=f"att{h}") for h in range(G)]

        # ---- Phase A: projections + rmsnorm + rope ----
        for c in range(NCH):
            cs = slice(c * CH, (c + 1) * CH)
            if c == 0:
                xt_c = xt0
            else:
                xt_c = xin.tile([P, KT, CH], bf16, tag="xt_c")
                for i in range(4):
                    nc.sync.dma_start(out=xt_c[:, 4 * i:4 * i + 4],
                                      in_=d_xt[c, :, 4 * i:4 * i + 4])
            # q heads + k: transposed projection [hd, s-chunk]
            for h in range(G + 1):
                is_q = h < G
                ps_q = p_pa.tile([P, CH], f32, tag="pa")
                for kt in range(KT):
                    lhs = wq_sb[:, kt, h, :] if is_q else wk_sb[:, kt, :]
                    nc.tensor.matmul(
                        ps_q, lhsT=lhs, rhs=xt_c[:, kt],
                        start=(kt == 0), stop=(kt == KT - 1),
                    )
                # rmsnorm: sumsq over hd via ones-matmul (M=128 -> broadcast rows)
                qsq = wnorm.tile([P, CH], bf16, tag="qsq")
                nc.scalar.activation(out=qsq, in_=ps_q, func=Act.Square)
                ss = p_red.tile([P, CH], f32, tag="red")
                nc.tensor.matmul(ss, lhsT=ones_bb, rhs=qsq, start=True, stop=True)
                ln = wnorm.tile([P, CH], f32, tag="ln")
                if is_q:
                    # rn = 1/sqrt(sumsq + HD*eps) == rmsnorm_scale * HD^-0.5
                    nc.scalar.activation(out=ln, in_=ss, func=Act.Ln,
                                         scale=1.0, bias=eps_q)
                else:
                    nc.scalar.activation(out=ln, in_=ss, func=Act.Ln,
                                         scale=1.0 / HD, bias=eps_k)
                rn = wnorm.tile([P, CH], f32, tag="rn")
                nc.scalar.activation(out=rn, in_=ln, func=Act.Exp, scale=-0.5)
                msb = wnorm.tile([P, CH], f32, tag="msb")
                nc.vector.tensor_scalar_mul(msb, rn, qsc_sb if is_q else ksc_sb)
                qs = work.tile([P, CH], bf16, tag="qs")
                nc.vector.tensor_mul(qs, ps_q, msb)
                # rope: out = qs*cos + swap(qs)*sin_signed (swap via PE permute)
                rot = p_sc.tile([P, CH], f32, tag="sc")
                nc.tensor.matmul(rot, lhsT=psw_sb, rhs=qs, start=True, stop=True)
                t1 = work.tile([P, CH], f32, tag="t1")
                nc.vector.tensor_mul(t1, qs, cos_sb[:, cs])
                u = work.tile([P, CH], f32, tag="u")
                nc.vector.tensor_mul(u, rot, sin_sb[:, cs])
                dst = qro[h] if is_q else kro
                nc.vector.tensor_add(dst[:, cs], t1, u)
            # v: transposed projection vT [hd, s-chunk] at N=512, then PE
            # transpose each 128-wide s-tile into natural [s, hd] layout
            vps = p_pa.tile([P, CH], f32, tag="pa")
            for kt in range(KT):
                nc.tensor.matmul(
                    vps, lhsT=wv_sb[:, kt], rhs=xt_c[:, kt],
                    start=(kt == 0), stop=(kt == KT - 1),
                )
            vtb = work.tile([P, CH], bf16, tag="qs")
            nc.vector.tensor_copy(vtb, vps)
            for st in range(4):
                vtp = p_sc.tile([P, P], bf16, tag="sc")
                nc.tensor.transpose(vtp, vtb[:, st * P:(st + 1) * P], ident_sb)
                nc.vector.tensor_copy(v_sb[:, 4 * c + st, :], vtp)

        # ---- Phase B: attention (scoresT layout, causal block skipping) ----
        for c in range(NCH):
            for h in range(G):
                cs = slice(c * CH, (c + 1) * CH)
                attps = p_acc.tile([P, CH], f32, tag="acc")
                csum = p_red.tile([P, CH], f32, tag="red")
                tmax = 4 * c + 4
                for t in range(tmax):
                    j = t - 4 * c
                    off = P * j if j > 0 else 0
                    sc = p_sc.tile([P, CH], f32, tag="sc")
                    nc.tensor.matmul(
                        sc[:, off:], lhsT=kro[:, t * P:(t + 1) * P],
                        rhs=qro[h][:, c * CH + off:(c + 1) * CH],
                        start=True, stop=True,
                    )
                    pb = work.tile([P, CH], bf16, tag="pb")
                    nc.scalar.activation(out=pb[:, off:], in_=sc[:, off:], func=Act.Exp)
                    if j >= 0:
                        # diagonal block: zero where sq < sk in the 128-col group
                        nc.vector.tensor_mul(pb[:, off:off + P], pb[:, off:off + P], tri_sb)
                    nc.tensor.matmul(csum[:, off:], lhsT=ones_bb, rhs=pb[:, off:],
                                     start=(t == 0), stop=(t == tmax - 1),
                                     skip_group_check=True)
                    nc.tensor.matmul(attps[:, off:], lhsT=v_sb[:, t, :], rhs=pb[:, off:],
                                     start=(t == 0), stop=(t == tmax - 1),
                                     skip_group_check=True)
                # normalize: att = attps / colsum (reciprocal on DVE, ~2ulp)
                rcp = wnorm.tile([P, CH], f32, tag="rn")
                scr = wnorm.tile([P, CH], f32, tag="ln")
                nc.vector.reciprocal_approx_accurate(out=rcp, in_=csum, scratch=scr)
                nc.vector.tensor_mul(att_sb[h][:, cs], attps, rcp)

            # ---- output projection for this chunk's s-tiles (PSUM -> DMA) ----
            for st in range(4 * c, 4 * c + 4):
                for mc in range(NCH):
                    ops = p_acc.tile([P, CH], f32, tag="acc")
                    for h in range(G):
                        nc.tensor.matmul(
                            ops, lhsT=att_sb[h][:, st * P:(st + 1) * P],
                            rhs=wo_sb[:, h, mc * CH:(mc + 1) * CH],
                            start=(h == 0), stop=(h == G - 1),
                        )
                    osb = work.tile([P, CH], f32, tag="osb")
                    nc.vector.tensor_copy(osb, ops)
                    nc.sync.dma_start(
                        out=d_out[st * P:(st + 1) * P, mc * CH:(mc + 1) * CH], in_=osb)

    # Pin every activation to the one table set that contains all functions
    # we use (exp/ln/copy/square), so the ACT engine never swaps tables.
    # Indices must stay aligned with act_info.json, so other sets are kept
    # in place but emptied (the pass then can't pick them).
    from concourse import bacc as bacc_mod
    orig_tables = bacc_mod.get_activation_tables
    target = "natural_log_exp_and_others"

    def unified_tables(arch):
        t = orig_tables(arch)
        assert target in t
        return {k: (v if k == target else set()) for k, v in t.items()}

    bacc_mod.get_activation_tables = unified_tables
    try:
        nc.compile()
    finally:
        bacc_mod.get_activation_tables = orig_tables
    return nc


def _get_nc():
    if "nc" not in _CACHE:
        _CACHE["nc"] = _build_nc()
    return _CACHE["nc"]


def _rope_tables():
    inv_ts = THETA ** (-np.arange(HD // 2, dtype=np.float64) / (HD // 2))
    ang = np.arange(S, dtype=np.float64)[None, :] * inv_ts[:, None]  # [64, S]
    cos64 = np.cos(ang)
    sin64 = np.sin(ang)
    cos_t = np.concatenate([cos64, cos64], 0).astype(np.float32)
    # rotate-then-multiply signs: top rows get -sin, bottom +sin
    sin_t = np.concatenate([-sin64, sin64], 0).astype(np.float32)
    return cos_t, sin_t


def kernel(x, wq, wk, wv, wo, q_scale, k_scale):
    bf = ml_dtypes.bfloat16
    x = np.asarray(x, np.float32)
    wq = np.asarray(wq, np.float32)
    wk = np.asarray(wk, np.float32)
    wv = np.asarray(wv, np.float32)
    wo = np.asarray(wo, np.float32)
    q_scale = np.asarray(q_scale, np.float32)
    k_scale = np.asarray(k_scale, np.float32)

    from concourse.bass_utils import run_bass_kernel_spmd

    nc = _get_nc()
    cos_t, sin_t = _rope_tables()
    half = P // 2
    psw = np.zeros((P, P), np.float32)
    psw[np.arange(half) + half, np.arange(half)] = 1.0
    psw[np.arange(half), np.arange(half) + half] = 1.0
    tri = (np.arange(P)[None, :] >= np.arange(P)[:, None]).astype(np.float32)

    in_maps = []
    for core in range(8):
        b, g = divmod(core, 4)
        in_maps.append({
            "xt": np.ascontiguousarray(
                x[b].T.reshape(KT, P, NCH, CH).transpose(2, 1, 0, 3)).astype(bf),
            "wq4": np.ascontiguousarray(
                wq[:, 4 * g:4 * g + 4, :].reshape(KT, P, G, HD).transpose(1, 0, 2, 3)).astype(bf),
            "wk1": np.ascontiguousarray(
                wk[:, g, :].reshape(KT, P, HD).transpose(1, 0, 2)).astype(bf),
            "wv1": np.ascontiguousarray(
                wv[:, g, :].reshape(KT, P, HD).transpose(1, 0, 2)).astype(bf),
            "wo4": np.ascontiguousarray(np.transpose(wo[4 * g:4 * g + 4], (1, 0, 2))).astype(bf),
            "qsc": q_scale.reshape(HD, 1),
            "ksc": k_scale.reshape(HD, 1),
            "cos_t": cos_t,
            "sin_t": sin_t,
            "psw": psw.astype(bf),
            "tri": tri.astype(bf),
            "ident": np.eye(P, dtype=np.float32).astype(bf),
        })

    res = run_bass_kernel_spmd(nc, in_maps, list(range(8)), **_RUN_KWARGS)
    _CACHE["last_res"] = res
    out = np.zeros((B, S, DM), np.float32)
    for core in range(8):
        out[core // 4] += res.results[core]["o_part"]
    return out


# revision 26
# speedup vs baseline: 1.0047x; 1.0047x over previous
"""GQA attention layer (B=2,S=2048,D=2048,H=16,KV=4,HD=128) on 8 trn2 cores.

Sharding: core = (b, g) for b in {0,1} (batch), g in {0..3} (kv group).
Each core computes q-heads 4g..4g+3 + kv head g for batch b, producing a
partial o-projection [S, D]; the host sums the 4 partials per batch.

Per-core kernel: everything in transposed layout (head_dim on partitions),
bf16 matmuls with fp32 accumulation, softmax without max-subtraction
(logits bounded after RMSNorm), causal block skipping. Partition-dim
reductions (rms-norm sum-of-squares, softmax denominator) via ones-matmul
with M=128 so the result is already broadcast across partitions;
reciprocals/rsqrt computed on ScalarE as exp(-a*ln(x)).
"""
import numpy as np
import ml_dtypes

B, S, DM = 2, 2048, 2048
H, KV, HD = 16, 4, 128
G = H // KV
THETA = 10000.0
EPS = 1e-6

P = 128         # partitions
CH = 512        # s-chunk (matmul N)
NCH = S // CH   # 4
KT = DM // P    # 16 contraction tiles
NST = S // P    # 16 s-tiles

_CACHE = {}
# extra kwargs for run_bass_kernel_spmd (test harness sets trace/tmpdir here)
_RUN_KWARGS = {}


def _build_nc():
    from concourse import bacc, mybir
    import concourse.tile as tile
    from contextlib import ExitStack

    f32 = mybir.dt.float32
    bf16 = mybir.dt.bfloat16
    Act = mybir.ActivationFunctionType

    nc = bacc.Bacc()
    d_xt = nc.declare_dram_parameter("xt", [NCH, P, KT, CH], bf16, isOutput=False)
    d_wq = nc.declare_dram_parameter("wq4", [P, KT, G, HD], bf16, isOutput=False)
    d_wk = nc.declare_dram_parameter("wk1", [P, KT, HD], bf16, isOutput=False)
    d_wv = nc.declare_dram_parameter("wv1", [P, KT, HD], bf16, isOutput=False)
    d_wo = nc.declare_dram_parameter("wo4", [HD, G, DM], bf16, isOutput=False)
    d_qs = nc.declare_dram_parameter("qsc", [HD, 1], f32, isOutput=False)
    d_ks = nc.declare_dram_parameter("ksc", [HD, 1], f32, isOutput=False)
    d_cos = nc.declare_dram_parameter("cos_t", [P, S], f32, isOutput=False)
    d_sin = nc.declare_dram_parameter("sin_t", [P, S], f32, isOutput=False)
    d_psw = nc.declare_dram_parameter("psw", [P, P], bf16, isOutput=False)
    d_tri = nc.declare_dram_parameter("tri", [P, P], bf16, isOutput=False)
    d_ident = nc.declare_dram_parameter("ident", [P, P], bf16, isOutput=False)
    d_out = nc.declare_dram_parameter("o_part", [S, DM], f32, isOutput=True)

    with tile.TileContext(nc) as tc, ExitStack() as ctx:
        const = ctx.enter_context(tc.tile_pool(name="const", bufs=1))
        xin = ctx.enter_context(tc.tile_pool(name="xin", bufs=2))
        work = ctx.enter_context(tc.tile_pool(name="work", bufs=4))
        wnorm = ctx.enter_context(tc.tile_pool(name="wnorm", bufs=2))
        # PSUM: 8 banks total, all pools alive all kernel so phases interleave
        p_pa = ctx.enter_context(tc.tile_pool(name="p_pa", bufs=3, space="PSUM"))
        p_sc = ctx.enter_context(tc.tile_pool(name="p_sc", bufs=2, space="PSUM"))
        p_red = ctx.enter_context(tc.tile_pool(name="p_red", bufs=1, space="PSUM"))
        p_acc = ctx.enter_context(tc.tile_pool(name="p_acc", bufs=2, space="PSUM"))

        # ---- persistent SBUF ----
        # first-needed data (wq k-tiles + first x chunk) interleaved on the
        # sync HWDGE ring; bulky later-needed tables go on the ACT ring.
        wq_sb = const.tile([P, KT, G, HD], bf16, tag="wq_sb")
        xt0 = xin.tile([P, KT, CH], bf16, tag="xt_c")
        for i in range(4):
            nc.sync.dma_start(out=wq_sb[:, 4 * i:4 * i + 4], in_=d_wq[:, 4 * i:4 * i + 4])
            nc.scalar.dma_start(out=xt0[:, 4 * i:4 * i + 4], in_=d_xt[0, :, 4 * i:4 * i + 4])
        wk_sb = const.tile([P, KT, HD], bf16, tag="wk_sb")
        nc.sync.dma_start(out=wk_sb, in_=d_wk[:])
        wv_sb = const.tile([P, KT, HD], bf16, tag="wv_sb")
        nc.sync.dma_start(out=wv_sb, in_=d_wv[:])
        qsc_sb = const.tile([HD, 1], f32, tag="qsc_sb")
        nc.sync.dma_start(out=qsc_sb, in_=d_qs[:])
        ksc_sb = const.tile([HD, 1], f32, tag="ksc_sb")
        nc.sync.dma_start(out=ksc_sb, in_=d_ks[:])
        psw_sb = const.tile([P, P], bf16, tag="psw_sb")
        nc.sync.dma_start(out=psw_sb, in_=d_psw[:])
        cos_sb = const.tile([P, S], f32, tag="cos_sb")
        nc.scalar.dma_start(out=cos_sb, in_=d_cos[:])
        sin_sb = const.tile([P, S], f32, tag="sin_sb")
        nc.scalar.dma_start(out=sin_sb, in_=d_sin[:])
        tri_sb = const.tile([P, P], bf16, tag="tri_sb")
        nc.scalar.dma_start(out=tri_sb, in_=d_tri[:])
        wo_sb = const.tile([P, G, DM], bf16, tag="wo_sb")
        nc.scalar.dma_start(out=wo_sb, in_=d_wo[:])
        ident_sb = const.tile([P, P], bf16, tag="ident_sb")
        nc.scalar.dma_start(out=ident_sb, in_=d_ident[:])
        ones_bb = const.tile([P, P], bf16, tag="ones_bb")
        nc.vector.memset(ones_bb, 1.0)
        eps_q = const.tile([P, 1], f32, tag="eps_q")
        nc.vector.memset(eps_q, float(HD * EPS))
        eps_k = const.tile([P, 1], f32, tag="eps_k")
        nc.vector.memset(eps_k, float(EPS))

        # roped q heads / k / v / normalized att, persistent
        qro = [const.tile([P, S], bf16, tag=f"qro{h}", name=f"qro{h}") for h in range(G)]
        kro = const.tile([P, S], bf16, tag="kro")
        v_sb = const.tile([P, NST, HD], bf16, tag="v_sb")
        att_sb = [const.tile([P, S], bf16, tag=f"att{h}", name=f"att{h}") for h in range(G)]

        # ---- Phase A: projections + rmsnorm + rope ----
        for c in range(NCH):
            cs = slice(c * CH, (c + 1) * CH)
            if c == 0:
                xt_c = xt0
            else:
                xt_c = xin.tile([P, KT, CH], bf16, tag="xt_c")
                for i in range(4):
                    nc.sync.dma_start(out=xt_c[:, 4 * i:4 * i + 4],
                                      in_=d_xt[c, :, 4 * i:4 * i + 4])
            # q heads + k: transposed projection [hd, s-chunk]
            for h in range(G + 1):
                is_q = h < G
                ps_q = p_pa.tile([P, CH], f32, tag="pa")
                for kt in range(KT):
                    lhs = wq_sb[:, kt, h, :] if is_q else wk_sb[:, kt, :]
                    nc.tensor.matmul(
                        ps_q, lhsT=lhs, rhs=xt_c[:, kt],
                        start=(kt == 0), stop=(kt == KT - 1),
                    )
                # rmsnorm: sumsq over hd via ones-matmul (M=128 -> broadcast rows)
                qsq = wnorm.tile([P, CH], bf16, tag="qsq")
                nc.scalar.activation(out=qsq, in_=ps_q, func=Act.Square)
                ss = p_red.tile([P, CH], f32, tag="red")
                nc.tensor.matmul(ss, lhsT=ones_bb, rhs=qsq, start=True, stop=True)
                ln = wnorm.tile([P, CH], f32, tag="ln")
                if is_q:
                    # rn = 1/sqrt(sumsq + HD*eps) == rmsnorm_scale * HD^-0.5
                    nc.scalar.activation(out=ln, in_=ss, func=Act.Ln,
                                         scale=1.0, bias=eps_q)
                else:
                    nc.scalar.activation(out=ln, in_=ss, func=Act.Ln,
                                         scale=1.0 / HD, bias=eps_k)
                rn = wnorm.tile([P, CH], f32, tag="rn")
                nc.scalar.activation(out=rn, in_=ln, func=Act.Exp, scale=-0.5)
                msb = wnorm.tile([P, CH], f32, tag="msb")
                nc.vector.tensor_scalar_mul(msb, rn, qsc_sb if is_q else ksc_sb)
                qs = work.tile([P, CH], bf16, tag="qs")
                nc.vector.tensor_mul(qs, ps_q, msb)
                # rope: out = qs*cos + swap(qs)*sin_signed (swap via PE permute)
                rot = p_sc.tile([P, CH], f32, tag="sc")
                nc.tensor.matmul(rot, lhsT=psw_sb, rhs=qs, start=True, stop=True)
                t1 = work.tile([P, CH], f32, tag="t1")
                nc.vector.tensor_mul(t1, qs, cos_sb[:, cs])
                u = work.tile([P, CH], f32, tag="u")
                nc.vector.tensor_mul(u, rot, sin_sb[:, cs])
                dst = qro[h] if is_q else kro
                nc.vector.tensor_add(dst[:, cs], t1, u)
            # v in natural [s, hd] layout
            for st in range(4):
                vps = p_pa.tile([P, HD], f32, tag="pa")
                for kt in range(KT):
                    nc.tensor.matmul(
                        vps, lhsT=xt_c[:, kt, st * P:(st + 1) * P], rhs=wv_sb[:, kt],
                        start=(kt == 0), stop=(kt == KT - 1),
                    )
                nc.vector.tensor_copy(v_sb[:, 4 * c + st, :], vps)

        # ---- Phase B: attention (scoresT layout, causal block skipping) ----
        for c in range(NCH):
            for h in range(G):
                cs = slice(c * CH, (c + 1) * CH)
                attps = p_acc.tile([P, CH], f32, tag="acc")
                csum = p_red.tile([P, CH], f32, tag="red")
                tmax = 4 * c + 4
                for t in range(tmax):
                    j = t - 4 * c
                    off = P * j if j > 0 else 0
                    sc = p_sc.tile([P, CH], f32, tag="sc")
                    nc.tensor.matmul(
                        sc[:, off:], lhsT=kro[:, t * P:(t + 1) * P],
                        rhs=qro[h][:, c * CH + off:(c + 1) * CH],
                        start=True, stop=True,
                    )
                    pb = work.tile([P, CH], bf16, tag="pb")
                    nc.scalar.activation(out=pb[:, off:], in_=sc[:, off:], func=Act.Exp)
                    if j >= 0:
                        # diagonal block: zero where sq < sk in the 128-col group
                        nc.vector.tensor_mul(pb[:, off:off + P], pb[:, off:off + P], tri_sb)
                    nc.tensor.matmul(csum[:, off:], lhsT=ones_bb, rhs=pb[:, off:],
                                     start=(t == 0), stop=(t == tmax - 1),
                                     skip_group_check=True)
                    nc.tensor.matmul(attps[:, off:], lhsT=v_sb[:, t, :], rhs=pb[:, off:],
                                     start=(t == 0), stop=(t == tmax - 1),
                                     skip_group_check=True)
                # normalize: att = attps / colsum (reciprocal on DVE, ~2ulp)
                rcp = wnorm.tile([P, CH], f32, tag="rn")
                scr = wnorm.tile([P, CH], f32, tag="ln")
                nc.vector.reciprocal_approx_accurate(out=rcp, in_=csum, scratch=scr)
                nc.vector.tensor_mul(att_sb[h][:, cs], attps, rcp)

            # ---- output projection for this chunk's s-tiles (PSUM -> DMA) ----
            for st in range(4 * c, 4 * c + 4):
                for mc in range(NCH):
                    ops = p_acc.tile([P, CH], f32, tag="acc")
                    for h in range(G):
                        nc.tensor.matmul(
                            ops, lhsT=att_sb[h][:, st * P:(st + 1) * P],
                            rhs=wo_sb[:, h, mc * CH:(mc + 1) * CH],
                            start=(h == 0), stop=(h == G - 1),
                        )
                    osb = work.tile([P, CH], f32, tag="osb")
                    nc.vector.tensor_copy(osb, ops)
                    nc.sync.dma_start(
                        out=d_out[st * P:(st + 1) * P, mc * CH:(mc + 1) * CH], in_=osb)

    # Pin every activation to the one table set that contains all functions
    # we use (exp/ln/copy/square), so the ACT engine never swaps tables.
    # Indices must stay aligned with act_info.json, so other sets are kept
    # in place but emptied (the pass then can't pick them).
    from concourse import bacc as bacc_mod
    orig_tables = bacc_mod.get_activation_tables
    target = "natural_log_exp_and_others"

    def unified_tables(arch):
        t = orig_tables(arch)
        assert target in t
        return {k: (v if k == target else set()) for k, v in t.items()}

    bacc_mod.get_activation_tables = unified_tables
    try:
        nc.compile()
    finally:
        bacc_mod.get_activation_tables = orig_tables
    return nc


def _get_nc():
    if "nc" not in _CACHE:
        _CACHE["nc"] = _build_nc()
    return _CACHE["nc"]


def _rope_tables():
    inv_ts = THETA ** (-np.arange(HD // 2, dtype=np.float64) / (HD // 2))
    ang = np.arange(S, dtype=np.float64)[None, :] * inv_ts[:, None]  # [64, S]
    cos64 = np.cos(ang)
    sin64 = np.sin(ang)
    cos_t = np.concatenate([cos64, cos64], 0).astype(np.float32)
    # rotate-then-multiply signs: top rows get -sin, bottom +sin
    sin_t = np.concatenate([-sin64, sin64], 0).astype(np.float32)
    return cos_t, sin_t


def kernel(x, wq, wk, wv, wo, q_scale, k_scale):
    bf = ml_dtypes.bfloat16
    x = np.asarray(x, np.float32)
    wq = np.asarray(wq, np.float32)
    wk = np.asarray(wk, np.float32)
    wv = np.asarray(wv, np.float32)
    wo = np.asarray(wo, np.float32)
    q_scale = np.asarray(q_scale, np.float32)
    k_scale = np.asarray(k_scale, np.float32)

    from concourse.bass_utils import run_bass_kernel_spmd

    nc = _get_nc()
    cos_t, sin_t = _rope_tables()
    half = P // 2
    psw = np.zeros((P, P), np.float32)
    psw[np.arange(half) + half, np.arange(half)] = 1.0
    psw[np.arange(half), np.arange(half) + half] = 1.0
    tri = (np.arange(P)[None, :] >= np.arange(P)[:, None]).astype(np.float32)

    in_maps = []
    for core in range(8):
        b, g = divmod(core, 4)
        in_maps.append({
            "xt": np.ascontiguousarray(
                x[b].T.reshape(KT, P, NCH, CH).transpose(2, 1, 0, 3)).astype(bf),
            "wq4": np.ascontiguousarray(
                wq[:, 4 * g:4 * g + 4, :].reshape(KT, P, G, HD).transpose(1, 0, 2, 3)).astype(bf),
            "wk1": np.ascontiguousarray(
                wk[:, g, :].reshape(KT, P, HD).transpose(1, 0, 2)).astype(bf),
            "wv1": np.ascontiguousarray(
                wv[:, g, :].reshape(KT, P, HD).transpose(1, 0, 2)).astype(bf),
            "wo4": np.ascontiguousarray(np.transpose(wo[4 * g:4 * g + 4], (1, 0, 2))).astype(bf),
            "qsc": q_scale.reshape(HD, 1),
            "ksc": k_scale.reshape(HD, 1),
            "cos_t": cos_t,
            "sin_t": sin_t,
            "psw": psw.astype(bf),
            "tri": tri.astype(bf),
            "ident": np.eye(P, dtype=np.float32).astype(bf),
        })

    res = run_bass_kernel_spmd(nc, in_maps, list(range(8)), **_RUN_KWARGS)
    _CACHE["last_res"] = res
    out = np.zeros((B, S, DM), np.float32)
    for core in range(8):
        out[core // 4] += res.results[core]["o_part"]
    return out


# revision 27
# speedup vs baseline: 1.1507x; 1.1453x over previous
"""GQA attention layer (B=2,S=2048,D=2048,H=16,KV=4,HD=128) on 8 trn2 cores.

Sharding: core = (b, g) for b in {0,1} (batch), g in {0..3} (kv group).
Each core computes q-heads 4g..4g+3 + kv head g for batch b, producing a
partial o-projection [S, D]; the host sums the 4 partials per batch.

Per-core kernel: everything in transposed layout (head_dim on partitions),
bf16 matmuls with fp32 accumulation, softmax without max-subtraction
(logits bounded after RMSNorm), causal block skipping. Partition-dim
reductions (rms-norm sum-of-squares, softmax denominator) via ones-matmul
with M=128 so the result is already broadcast across partitions;
reciprocals/rsqrt computed on ScalarE as exp(-a*ln(x)).
"""
import numpy as np
import ml_dtypes

B, S, DM = 2, 2048, 2048
H, KV, HD = 16, 4, 128
G = H // KV
THETA = 10000.0
EPS = 1e-6

P = 128         # partitions
CH = 512        # s-chunk (matmul N)
NCH = S // CH   # 4
KT = DM // P    # 16 contraction tiles
NST = S // P    # 16 s-tiles

_CACHE = {}
# extra kwargs for run_bass_kernel_spmd (test harness sets trace/tmpdir here)
_RUN_KWARGS = {}


def _build_nc():
    from concourse import bacc, mybir
    import concourse.tile as tile
    from contextlib import ExitStack

    f32 = mybir.dt.float32
    bf16 = mybir.dt.bfloat16
    Act = mybir.ActivationFunctionType

    nc = bacc.Bacc()
    d_xt = nc.declare_dram_parameter("xt", [NCH, P, KT, CH], bf16, isOutput=False)
    d_wq = nc.declare_dram_parameter("wq4", [P, KT, G, HD], bf16, isOutput=False)
    d_wk = nc.declare_dram_parameter("wk1", [P, KT, HD], bf16, isOutput=False)
    d_wv = nc.declare_dram_parameter("wv1", [P, KT, HD], bf16, isOutput=False)
    d_wo = nc.declare_dram_parameter("wo4", [HD, G, DM], bf16, isOutput=False)
    d_qs = nc.declare_dram_parameter("qsc", [HD, 1], f32, isOutput=False)
    d_ks = nc.declare_dram_parameter("ksc", [HD, 1], f32, isOutput=False)
    d_cos = nc.declare_dram_parameter("cos_t", [P, S], f32, isOutput=False)
    d_sin = nc.declare_dram_parameter("sin_t", [P, S], f32, isOutput=False)
    d_psw = nc.declare_dram_parameter("psw", [P, P], bf16, isOutput=False)
    d_tri = nc.declare_dram_parameter("tri", [P, P], bf16, isOutput=False)
    d_out = nc.declare_dram_parameter("o_part", [S, DM], f32, isOutput=True)

    with tile.TileContext(nc) as tc, ExitStack() as ctx:
        const = ctx.enter_context(tc.tile_pool(name="const", bufs=1))
        xin = ctx.enter_context(tc.tile_pool(name="xin", bufs=2))
        work = ctx.enter_context(tc.tile_pool(name="work", bufs=4))
        wnorm = ctx.enter_context(tc.tile_pool(name="wnorm", bufs=2))
        # PSUM: 8 banks total, all pools alive all kernel so phases interleave
        p_pa = ctx.enter_context(tc.tile_pool(name="p_pa", bufs=3, space="PSUM"))
        p_sc = ctx.enter_context(tc.tile_pool(name="p_sc", bufs=2, space="PSUM"))
        p_red = ctx.enter_context(tc.tile_pool(name="p_red", bufs=1, space="PSUM"))
        p_acc = ctx.enter_context(tc.tile_pool(name="p_acc", bufs=2, space="PSUM"))

        # ---- persistent SBUF ----
        # first-needed data (wq k-tiles + first x chunk) interleaved on the
        # sync HWDGE ring; bulky later-needed tables go on the ACT ring.
        wq_sb = const.tile([P, KT, G, HD], bf16, tag="wq_sb")
        xt0 = xin.tile([P, KT, CH], bf16, tag="xt_c")
        for i in range(4):
            nc.sync.dma_start(out=wq_sb[:, 4 * i:4 * i + 4], in_=d_wq[:, 4 * i:4 * i + 4])
            nc.sync.dma_start(out=xt0[:, 4 * i:4 * i + 4], in_=d_xt[0, :, 4 * i:4 * i + 4])
        wk_sb = const.tile([P, KT, HD], bf16, tag="wk_sb")
        nc.sync.dma_start(out=wk_sb, in_=d_wk[:])
        wv_sb = const.tile([P, KT, HD], bf16, tag="wv_sb")
        nc.sync.dma_start(out=wv_sb, in_=d_wv[:])
        qsc_sb = const.tile([HD, 1], f32, tag="qsc_sb")
        nc.sync.dma_start(out=qsc_sb, in_=d_qs[:])
        ksc_sb = const.tile([HD, 1], f32, tag="ksc_sb")
        nc.sync.dma_start(out=ksc_sb, in_=d_ks[:])
        psw_sb = const.tile([P, P], bf16, tag="psw_sb")
        nc.sync.dma_start(out=psw_sb, in_=d_psw[:])
        cos_sb = const.tile([P, S], f32, tag="cos_sb")
        nc.scalar.dma_start(out=cos_sb, in_=d_cos[:])
        sin_sb = const.tile([P, S], f32, tag="sin_sb")
        nc.scalar.dma_start(out=sin_sb, in_=d_sin[:])
        tri_sb = const.tile([P, P], bf16, tag="tri_sb")
        nc.scalar.dma_start(out=tri_sb, in_=d_tri[:])
        wo_sb = const.tile([P, G, DM], bf16, tag="wo_sb")
        nc.scalar.dma_start(out=wo_sb, in_=d_wo[:])
        ones_bb = const.tile([P, P], bf16, tag="ones_bb")
        nc.vector.memset(ones_bb, 1.0)
        eps_q = const.tile([P, 1], f32, tag="eps_q")
        nc.vector.memset(eps_q, float(HD * EPS))
        eps_k = const.tile([P, 1], f32, tag="eps_k")
        nc.vector.memset(eps_k, float(EPS))

        # roped q heads / k / v / normalized att, persistent
        qro = [const.tile([P, S], bf16, tag=f"qro{h}", name=f"qro{h}") for h in range(G)]
        kro = const.tile([P, S], bf16, tag="kro")
        v_sb = const.tile([P, NST, HD], bf16, tag="v_sb")
        att_sb = [const.tile([P, S], bf16, tag=f"att{h}", name=f"att{h}") for h in range(G)]

        # ---- Phase A: projections + rmsnorm + rope ----
        for c in range(NCH):
            cs = slice(c * CH, (c + 1) * CH)
            if c == 0:
                xt_c = xt0
            else:
                xt_c = xin.tile([P, KT, CH], bf16, tag="xt_c")
                for i in range(4):
                    nc.sync.dma_start(out=xt_c[:, 4 * i:4 * i + 4],
                                      in_=d_xt[c, :, 4 * i:4 * i + 4])
            # q heads + k: transposed projection [hd, s-chunk]
            for h in range(G + 1):
                is_q = h < G
                ps_q = p_pa.tile([P, CH], f32, tag="pa")
                for kt in range(KT):
                    lhs = wq_sb[:, kt, h, :] if is_q else wk_sb[:, kt, :]
                    nc.tensor.matmul(
                        ps_q, lhsT=lhs, rhs=xt_c[:, kt],
                        start=(kt == 0), stop=(kt == KT - 1),
                    )
                # rmsnorm: sumsq over hd via ones-matmul (M=128 -> broadcast rows)
                qsq = wnorm.tile([P, CH], bf16, tag="qsq")
                nc.scalar.activation(out=qsq, in_=ps_q, func=Act.Square)
                ss = p_red.tile([P, CH], f32, tag="red")
                nc.tensor.matmul(ss, lhsT=ones_bb, rhs=qsq, start=True, stop=True)
                ln = wnorm.tile([P, CH], f32, tag="ln")
                if is_q:
                    # rn = 1/sqrt(sumsq + HD*eps) == rmsnorm_scale * HD^-0.5
                    nc.scalar.activation(out=ln, in_=ss, func=Act.Ln,
                                         scale=1.0, bias=eps_q)
                else:
                    nc.scalar.activation(out=ln, in_=ss, func=Act.Ln,
                                         scale=1.0 / HD, bias=eps_k)
                rn = wnorm.tile([P, CH], f32, tag="rn")
                nc.scalar.activation(out=rn, in_=ln, func=Act.Exp, scale=-0.5)
                msb = wnorm.tile([P, CH], f32, tag="msb")
                nc.vector.tensor_scalar_mul(msb, rn, qsc_sb if is_q else ksc_sb)
                qs = work.tile([P, CH], bf16, tag="qs")
                nc.vector.tensor_mul(qs, ps_q, msb)
                # rope: out = qs*cos + swap(qs)*sin_signed (swap via PE permute)
                rot = p_sc.tile([P, CH], f32, tag="sc")
                nc.tensor.matmul(rot, lhsT=psw_sb, rhs=qs, start=True, stop=True)
                t1 = work.tile([P, CH], f32, tag="t1")
                nc.vector.tensor_mul(t1, qs, cos_sb[:, cs])
                u = work.tile([P, CH], f32, tag="u")
                nc.vector.tensor_mul(u, rot, sin_sb[:, cs])
                dst = qro[h] if is_q else kro
                nc.vector.tensor_add(dst[:, cs], t1, u)
            # v in natural [s, hd] layout
            for st in range(4):
                vps = p_pa.tile([P, HD], f32, tag="pa")
                for kt in range(KT):
                    nc.tensor.matmul(
                        vps, lhsT=xt_c[:, kt, st * P:(st + 1) * P], rhs=wv_sb[:, kt],
                        start=(kt == 0), stop=(kt == KT - 1),
                    )
                nc.vector.tensor_copy(v_sb[:, 4 * c + st, :], vps)

        # ---- Phase B: attention (scoresT layout, causal block skipping) ----
        for c in range(NCH):
            for h in range(G):
                cs = slice(c * CH, (c + 1) * CH)
                attps = p_acc.tile([P, CH], f32, tag="acc")
                csum = p_red.tile([P, CH], f32, tag="red")
                tmax = 4 * c + 4
                for t in range(tmax):
                    j = t - 4 * c
                    off = P * j if j > 0 else 0
                    sc = p_sc.tile([P, CH], f32, tag="sc")
                    nc.tensor.matmul(
                        sc[:, off:], lhsT=kro[:, t * P:(t + 1) * P],
                        rhs=qro[h][:, c * CH + off:(c + 1) * CH],
                        start=True, stop=True,
                    )
                    pb = work.tile([P, CH], bf16, tag="pb")
                    nc.scalar.activation(out=pb[:, off:], in_=sc[:, off:], func=Act.Exp)
                    if j >= 0:
                        # diagonal block: zero where sq < sk in the 128-col group
                        nc.vector.tensor_mul(pb[:, off:off + P], pb[:, off:off + P], tri_sb)
                    nc.tensor.matmul(csum[:, off:], lhsT=ones_bb, rhs=pb[:, off:],
                                     start=(t == 0), stop=(t == tmax - 1),
                                     skip_group_check=True)
                    nc.tensor.matmul(attps[:, off:], lhsT=v_sb[:, t, :], rhs=pb[:, off:],
                                     start=(t == 0), stop=(t == tmax - 1),
                                     skip_group_check=True)
                # normalize: att = attps / colsum (reciprocal on DVE, ~2ulp)
                rcp = wnorm.tile([P, CH], f32, tag="rn")
                scr = wnorm.tile([P, CH], f32, tag="ln")
                nc.vector.reciprocal_approx_accurate(out=rcp, in_=csum, scratch=scr)
                nc.vector.tensor_mul(att_sb[h][:, cs], attps, rcp)

            # ---- output projection for this chunk's s-tiles (PSUM -> DMA) ----
            for st in range(4 * c, 4 * c + 4):
                for mc in range(NCH):
                    ops = p_acc.tile([P, CH], f32, tag="acc")
                    for h in range(G):
                        nc.tensor.matmul(
                            ops, lhsT=att_sb[h][:, st * P:(st + 1) * P],
                            rhs=wo_sb[:, h, mc * CH:(mc + 1) * CH],
                            start=(h == 0), stop=(h == G - 1),
                        )
                    osb = work.tile([P, CH], f32, tag="osb")
                    nc.vector.tensor_copy(osb, ops)
                    nc.sync.dma_start(
                        out=d_out[st * P:(st + 1) * P, mc * CH:(mc + 1) * CH], in_=osb)

    # Pin every activation to the one table set that contains all functions
    # we use (exp/ln/copy/square), so the ACT engine never swaps tables.
    # Indices must stay aligned with act_info.json, so other sets are kept
    # in place but emptied (the pass then can't pick them).
    from concourse import bacc as bacc_mod
    orig_tables = bacc_mod.get_activation_tables
    target = "natural_log_exp_and_others"

    def unified_tables(arch):
        t = orig_tables(arch)
        assert target in t
        return {k: (v if k == target else set()) for k, v in t.items()}

    bacc_mod.get_activation_tables = unified_tables
    try:
        nc.compile()
    finally:
        bacc_mod.get_activation_tables = orig_tables
    return nc


def _get_nc():
    if "nc" not in _CACHE:
        _CACHE["nc"] = _build_nc()
    return _CACHE["nc"]


def _rope_tables():
    inv_ts = THETA ** (-np.arange(HD // 2, dtype=np.float64) / (HD // 2))
    ang = np.arange(S, dtype=np.float64)[None, :] * inv_ts[:, None]  # [64, S]
    cos64 = np.cos(ang)
    sin64 = np.sin(ang)
    cos_t = np.concatenate([cos64, cos64], 0).astype(np.float32)
    # rotate-then-multiply signs: top rows get -sin, bottom +sin
    sin_t = np.concatenate([-sin64, sin64], 0).astype(np.float32)
    return cos_t, sin_t


def kernel(x, wq, wk, wv, wo, q_scale, k_scale):
    bf = ml_dtypes.bfloat16
    x = np.asarray(x, np.float32)
    wq = np.asarray(wq, np.float32)
    wk = np.asarray(wk, np.float32)
    wv = np.asarray(wv, np.float32)
    wo = np.asarray(wo, np.float32)
    q_scale = np.asarray(q_scale, np.float32)
    k_scale = np.asarray(k_scale, np.float32)

    from concourse.bass_utils import run_bass_kernel_spmd

    nc = _get_nc()
    cos_t, sin_t = _rope_tables()
    half = P // 2
    psw = np.zeros((P, P), np.float32)
    psw[np.arange(half) + half, np.arange(half)] = 1.0
    psw[np.arange(half), np.arange(half) + half] = 1.0
    tri = (np.arange(P)[None, :] >= np.arange(P)[:, None]).astype(np.float32)

    in_maps = []
    for core in range(8):
        b, g = divmod(core, 4)
        in_maps.append({
            "xt": np.ascontiguousarray(
                x[b].T.reshape(KT, P, NCH, CH).transpose(2, 1, 0, 3)).astype(bf),
            "wq4": np.ascontiguousarray(
                wq[:, 4 * g:4 * g + 4, :].reshape(KT, P, G, HD).transpose(1, 0, 2, 3)).astype(bf),
            "wk1": np.ascontiguousarray(
                wk[:, g, :].reshape(KT, P, HD).transpose(1, 0, 2)).astype(bf),
            "wv1": np.ascontiguousarray(
                wv[:, g, :].reshape(KT, P, HD).transpose(1, 0, 2)).astype(bf),
            "wo4": np.ascontiguousarray(np.transpose(wo[4 * g:4 * g + 4], (1, 0, 2))).astype(bf),
            "qsc": q_scale.reshape(HD, 1),
            "ksc": k_scale.reshape(HD, 1),
            "cos_t": cos_t,
            "sin_t": sin_t,
            "psw": psw.astype(bf),
            "tri": tri.astype(bf),
        })

    res = run_bass_kernel_spmd(nc, in_maps, list(range(8)), **_RUN_KWARGS)
    _CACHE["last_res"] = res
    out = np.zeros((B, S, DM), np.float32)
    for core in range(8):
        out[core // 4] += res.results[core]["o_part"]
    return out
